# revision 20
# baseline (speedup 1.0000x reference)
"""DeepseekV3 top-k router kernel for 8x Trainium2 NeuronCores.

Strategy:
  - Token dim (8192) sharded 8 ways; router weight replicated per core.
  - logits = hidden @ W.T computed as an fp16 hi/lo split matmul (3 PE passes:
    hi*Whi -> psum_hi; (lo*Whi + hi*Wlo) -> psum_lo, lo parts pre-scaled by
    2^11) giving full-fp32-quality logits at the PE's 2-byte streaming rate.
  - sigmoid + grouped top-k + gather + normalization run on ACT/DVE per
    128-token tile, overlapped with the next tile's matmuls.
Host side packs hidden into transposed, fp16-split, per-core tiled layout and
replicates W/biases; device returns top-8 indices (int32) + weights (f32).
"""

import numpy as np
import ml_dtypes

import concourse.bacc as bacc
import concourse.mybir as mybir
from concourse.tile import TileContext
from concourse import bass_utils

H = 7168
E = 256
T = 8192
NCORES = 8
TLOC = T // NCORES          # 1024 tokens per core
MT = 128                    # tokens per tile (PSUM partition dim)
NM = TLOC // MT             # 8 token tiles per core
KT = H // 128               # 56 contraction tiles
XC = 7                      # x stream chunks per token tile
KX = KT // XC               # 8 k-tiles per x chunk
W_CH = [2, 2] + [4] * 13    # W resident chunk sizes (k-tiles)
WC = len(W_CH)
W_OFF = [sum(W_CH[:i]) for i in range(WC)]
X0_CH = [2, 6] + [8] * 6    # m=0 x chunk sizes
X_CH = [8] * 7              # m>0 x chunk sizes
TOP_K = 8
N_GROUP = 8
TOPK_GROUP = 4
EG = E // N_GROUP           # 32 experts per group
SCALE = 2.5
SC = 2048.0                 # 2^11 lo-part pre-scale

f32 = mybir.dt.float32
f16 = mybir.dt.float16
u32 = mybir.dt.uint32
i32 = mybir.dt.int32
AOT = mybir.AluOpType
ACTF = mybir.ActivationFunctionType

_PROG = None


def _build(stage="full"):
    nc = bacc.Bacc(trn_type="TRN2")
    X = nc.dram_tensor("x", [NM, 128, KT, 2 * MT], f16, kind="ExternalInput")
    Wd = nc.dram_tensor("w", [128, KT, 2 * E], f16, kind="ExternalInput")
    C = nc.dram_tensor("c", [128, 3 * E], f32, kind="ExternalInput")
    OIDX = nc.dram_tensor("oidx", [TLOC, TOP_K], i32, kind="ExternalOutput")
    OW = nc.dram_tensor("ow", [TLOC, TOP_K], f32, kind="ExternalOutput")
    DBG = None
    if stage != "full":
        DBG = nc.dram_tensor("dbg", [TLOC, E], f32, kind="ExternalOutput")

    with TileContext(nc) as tc:
        with (
            tc.tile_pool(name="const", bufs=1) as cpool,
            tc.tile_pool(name="xs", bufs=2) as xpool,
            tc.tile_pool(name="s2", bufs=2) as s2,
            tc.tile_pool(name="psum", bufs=2, space="PSUM") as pspool,
            tc.tile_pool(name="psum1", bufs=1, space="PSUM") as pspool1,
        ):
            c_sb = cpool.tile([128, 3 * E], f32, name="c_sb")
            nc.sync.dma_start(c_sb[:, :], C[:, :])
            b_rep = c_sb[:, 0:E]
            eb_rep = c_sb[:, E:2 * E]
            iota = c_sb[:, 2 * E:3 * E]

            def load_x(m, hf, k0, nk):
                xt = xpool.tile([128, nk * 2 * MT], f16, tag=f"x{hf}",
                                name=f"x{hf}_{m}")
                nc.sync.dma_start(
                    xt.rearrange("p (k t) -> p k t", k=nk),
                    X[m, :, k0:k0 + nk, :],
                )
                return xt

            # Startup DMAs interleaved in the order the k-loop consumes them
            # (the HWDGE queues drain FIFO, so W must not be queued ahead of
            # the first token tile's activations).
            w_sbs = [None] * WC
            x0_off = [sum(X0_CH[:i]) for i in range(len(X0_CH))]
            x_off = [sum(X_CH[:i]) for i in range(len(X_CH))]

            # m0 and m1 run interleaved so the PE has two tiles of work while
            # the (DMA-bound) W load streams in; later tiles go single-file.
            groups = [[0, 1]] + [[m] for m in range(2, NM)]

            # Startup DMAs interleaved in k-consumption order (HWDGE FIFO).
            startup = []
            wi = 0
            xi = {0: 0, 1: 0, 2: 0}
            for k in range(KT):
                while wi < WC and W_OFF[wi] <= k:
                    startup.append(("w", 0, wi)); wi += 1
                while xi[0] < len(X0_CH) and x0_off[xi[0]] <= k:
                    startup.append(("x", 0, xi[0])); xi[0] += 1
                for mm in (1,):
                    while xi[mm] < len(X_CH) and x_off[xi[mm]] <= k:
                        startup.append(("x", mm, xi[mm])); xi[mm] += 1
            x_t = {0: [None] * len(X0_CH), 1: [None] * len(X_CH)}
            for kind, m, i in startup:
                if kind == "w":
                    nk = W_CH[i]
                    wt = cpool.tile([128, nk * 2 * E], f16, name=f"w_sb{i}")
                    nc.sync.dma_start(
                        wt.rearrange("p (k e) -> p k e", k=nk),
                        Wd[:, W_OFF[i]:W_OFF[i] + nk, :],
                    )
                    w_sbs[i] = wt
                else:
                    off = x0_off if m == 0 else x_off
                    ch = X0_CH if m == 0 else X_CH
                    x_t[m][i] = load_x(m, i, off[i], ch[i])

            wmap = []
            for ci, n in enumerate(W_CH):
                wmap += [(ci, j) for j in range(n)]

            def xmap_for(ch):
                mp = []
                for ci, n in enumerate(ch):
                    mp += [(ci, j) for j in range(n)]
                return mp

            def stage2(m, ps_hi, ps_lo):
                lg = s2.tile([128, E], f32, tag="lg", name=f"lg{m}")
                nc.vector.tensor_scalar(lg[:, :], ps_lo[:, :], 1.0 / SC, None,
                                        op0=AOT.mult)
                nc.vector.tensor_add(lg[:, :], lg[:, :], ps_hi[:, :])
                nc.vector.tensor_add(lg[:, :], lg[:, :], b_rep)
                s = s2.tile([128, E], f32, tag="s", name=f"s{m}")
                nc.scalar.activation(s[:, :], lg[:, :], ACTF.Sigmoid)
                sfc = s2.tile([128, E], f32, tag="sfc", name=f"sfc{m}")
                nc.vector.tensor_add(sfc[:, :], s[:, :], eb_rep)

                gmax = s2.tile([128, 8 * N_GROUP], f32, tag="gmax", name=f"gmax{m}")
                for g in range(N_GROUP):
                    nc.vector.max(out=gmax[:, g * 8:(g + 1) * 8],
                                  in_=sfc[:, g * EG:(g + 1) * EG])
                gm3 = gmax.rearrange("p (g c) -> p g c", c=8)
                gs = s2.tile([128, N_GROUP], f32, tag="gs", name=f"gs{m}")
                nc.vector.tensor_add(gs.unsqueeze(2), gm3[:, :, 0:1], gm3[:, :, 1:2])
                g8 = s2.tile([128, 8], f32, tag="g8", name=f"g8{m}")
                nc.vector.max(out=g8[:, :], in_=gs[:, :])
                gmask = s2.tile([128, N_GROUP], f32, tag="gmask", name=f"gmask{m}")
                nc.vector.tensor_scalar(gmask[:, :], gs[:, :],
                                        g8[:, TOPK_GROUP - 1:TOPK_GROUP], None,
                                        op0=AOT.is_ge)
                # top-8 values from the masked per-group maxima (64 wide) --
                # same fp32 values as a 256-wide masked scan, but cheaper.
                gmm = s2.tile([128, 8 * N_GROUP], f32, tag="gmm", name=f"gmm{m}")
                nc.vector.tensor_mul(
                    gmm.rearrange("p (g c) -> p g c", c=8),
                    gm3,
                    gmask.unsqueeze(2).to_broadcast([128, N_GROUP, 8]),
                )
                m8 = s2.tile([128, 8], f32, tag="m8", name=f"m8{m}")
                nc.vector.max(out=m8[:, :], in_=gmm[:, :])

                # Gather unbiased scores by matching top-8 values in sfc
                # (f32 value-match; int converts + ttr hang on hw).
                w8 = s2.tile([128, 8], f32, tag="w8", name=f"w8{m}")
                eqa = s2.tile([128, TOP_K * E], f32, tag="eqa", name=f"eqa{m}")
                scra = s2.tile([128, TOP_K * E], f32, tag="scra", name=f"scra{m}")
                for j in range(TOP_K):
                    nc.vector.tensor_scalar(eqa[:, j * E:(j + 1) * E],
                                            sfc[:, :], m8[:, j:j + 1], None,
                                            op0=AOT.is_equal)
                nc.vector.tensor_mul(
                    scra.rearrange("p (j e) -> p j e", j=TOP_K),
                    eqa.rearrange("p (j e) -> p j e", j=TOP_K),
                    s.unsqueeze(1).to_broadcast([128, TOP_K, E]),
                )
                nc.vector.tensor_reduce(w8[:, :],
                                        scra.rearrange("p (j e) -> p j e", j=TOP_K),
                                        axis=mybir.AxisListType.X, op=AOT.add)
                rs = s2.tile([128, 1], f32, tag="rs", name=f"rs{m}")
                nc.vector.tensor_reduce(rs[:, :], w8[:, :],
                                        axis=mybir.AxisListType.X, op=AOT.add)
                rc = s2.tile([128, 1], f32, tag="rc", name=f"rc{m}")
                nc.vector.reciprocal(rc[:, :], rs[:, :])
                wo = s2.tile([128, 8], f32, tag="wo", name=f"wo{m}")
                nc.vector.tensor_scalar(wo[:, :], w8[:, :], rc[:, 0:1], SCALE,
                                        op0=AOT.mult, op1=AOT.mult)
                nc.sync.dma_start(OW[m * MT:(m + 1) * MT, :], wo[:, :])

                # index extraction (off the critical path -- output only)
                masked = s2.tile([128, E], f32, tag="masked", name=f"masked{m}")
                nc.vector.tensor_mul(
                    masked.rearrange("p (g c) -> p g c", c=EG),
                    sfc.rearrange("p (g c) -> p g c", c=EG),
                    gmask.unsqueeze(2).to_broadcast([128, N_GROUP, EG]),
                )
                i8 = s2.tile([128, 8], u32, tag="i8", name=f"i8{m}")
                nc.vector.max_index(out=i8[:, :], in_max=m8[:, :],
                                    in_values=masked[:, :])
                nc.sync.dma_start(OIDX[m * MT:(m + 1) * MT, :], i8.bitcast(i32))

            for group in groups:
                pss = {}
                for gi, m in enumerate(group):
                    pool = pspool if gi == 0 else pspool1
                    pss[m] = (
                        pool.tile([128, E], f32, tag=f"ps_hi{gi}", name=f"ps_hi{m}"),
                        pool.tile([128, E], f32, tag=f"ps_lo{gi}", name=f"ps_lo{m}"),
                    )
                xts = {}
                for m in group:
                    if m in x_t:
                        xts[m] = (x_t[m], xmap_for(X0_CH if m == 0 else X_CH))
                    else:
                        xts[m] = ([load_x(m, hf, x_off[hf], X_CH[hf])
                                   for hf in range(len(X_CH))], xmap_for(X_CH))
                for k in range(KT):
                    wc, kw = wmap[k]
                    wt = w_sbs[wc]
                    wh = wt[:, kw * 2 * E: kw * 2 * E + E]
                    wl = wt[:, kw * 2 * E + E: (kw + 1) * 2 * E]
                    for m in group:
                        xtl, xmp = xts[m]
                        xc, kl = xmp[k]
                        xt = xtl[xc]
                        xh = xt[:, kl * 2 * MT: kl * 2 * MT + MT]
                        xl = xt[:, kl * 2 * MT + MT: (kl + 1) * 2 * MT]
                        ps_hi, ps_lo = pss[m]
                        nc.tensor.matmul(ps_hi[:, :], xh, wh,
                                         start=(k == 0), stop=(k == KT - 1))
                        nc.tensor.matmul(ps_lo[:, :], xl, wh,
                                         start=(k == 0), stop=False)
                        nc.tensor.matmul(ps_lo[:, :], xh, wl,
                                         start=False, stop=(k == KT - 1))
                for m in group:
                    stage2(m, *pss[m])

    nc.finalize()
    return nc


def _pack_hidden(x_shard: np.ndarray) -> np.ndarray:
    """[TLOC, H] f32 -> [NM, 128, KT, 2*MT] f16 (hi | scaled lo per k-tile)."""
    xT = np.ascontiguousarray(x_shard.T)               # [H, TLOC]
    xh = xT.astype(np.float16)
    xl = ((xT - xh.astype(np.float32)) * SC).astype(np.float16)
    out = np.empty((NM, 128, KT, 2 * MT), np.float16)
    # xh[k*128+p, m*128+t] -> out[m, p, k, t]
    xh4 = xh.reshape(KT, 128, NM, MT).transpose(2, 1, 0, 3)
    xl4 = xl.reshape(KT, 128, NM, MT).transpose(2, 1, 0, 3)
    out[:, :, :, :MT] = xh4
    out[:, :, :, MT:] = xl4
    return np.ascontiguousarray(out)


def _pack_w(W: np.ndarray) -> np.ndarray:
    """[E, H] f32 -> [128, KT, 2*E] f16 (hi | scaled lo)."""
    wT = np.ascontiguousarray(W.T)                     # [H, E]
    wh = wT.astype(np.float16)
    wl = ((wT - wh.astype(np.float32)) * SC).astype(np.float16)
    out = np.empty((128, KT, 2 * E), np.float16)
    out[:, :, :E] = wh.reshape(KT, 128, E).transpose(1, 0, 2)
    out[:, :, E:] = wl.reshape(KT, 128, E).transpose(1, 0, 2)
    return np.ascontiguousarray(out)


def kernel(hidden_states, W, b, e_score_correction_bias):
    global _PROG
    hidden_states = np.asarray(hidden_states, np.float32)
    W = np.asarray(W, np.float32)
    b = np.asarray(b, np.float32)
    eb = np.asarray(e_score_correction_bias, np.float32)

    if _PROG is None:
        _PROG = _build()
    nc = _PROG

    wp = _pack_w(W)
    consts = np.empty((128, 3 * E), np.float32)
    consts[:, 0:E] = b[None, :]
    consts[:, E:2 * E] = eb[None, :]
    consts[:, 2 * E:3 * E] = np.arange(E, dtype=np.float32)[None, :]

    in_maps = []
    for c in range(NCORES):
        shard = hidden_states[c * TLOC:(c + 1) * TLOC]
        in_maps.append({"x": _pack_hidden(shard), "w": wp, "c": consts})

    res = bass_utils.run_bass_kernel_spmd(nc, in_maps, core_ids=list(range(NCORES)))

    idx = np.concatenate([res.results[c]["oidx"] for c in range(NCORES)], axis=0)
    wts = np.concatenate([res.results[c]["ow"] for c in range(NCORES)], axis=0)
    return idx.astype(np.int32), wts.astype(np.float32)


# revision 23
# speedup vs baseline: 1.0176x; 1.0176x over previous
"""DeepseekV3 top-k router kernel for 8x Trainium2 NeuronCores.

Strategy:
  - Token dim (8192) sharded 8 ways; router weight replicated per core.
  - logits = hidden @ W.T computed as an fp16 hi/lo split matmul (3 PE passes:
    hi*Whi -> psum_hi; (lo*Whi + hi*Wlo) -> psum_lo, lo parts pre-scaled by
    2^11) giving full-fp32-quality logits at the PE's 2-byte streaming rate.
  - sigmoid + grouped top-k + gather + normalization run on ACT/DVE per
    128-token tile, overlapped with the next tile's matmuls.
Host side packs hidden into transposed, fp16-split, per-core tiled layout and
replicates W/biases; device returns top-8 indices (int32) + weights (f32).
"""

import numpy as np

import concourse.bacc as bacc
import concourse.mybir as mybir
from concourse.tile import TileContext
from concourse import bass_utils

H = 7168
E = 256
T = 8192
NCORES = 8
TLOC = T // NCORES          # 1024 tokens per core
MT = 128                    # tokens per tile (PSUM partition dim)
NM = TLOC // MT             # 8 token tiles per core
KT = H // 128               # 56 contraction tiles
XC = 7                      # x stream chunks per token tile
KX = KT // XC               # 8 k-tiles per x chunk
W_CH = [2, 2] + [4] * 13    # W resident chunk sizes (k-tiles)
WC = len(W_CH)
W_OFF = [sum(W_CH[:i]) for i in range(WC)]
X0_CH = [2, 6] + [8] * 6    # m=0 x chunk sizes
X_CH = [8] * 7              # m>0 x chunk sizes
TOP_K = 8
N_GROUP = 8
TOPK_GROUP = 4
EG = E // N_GROUP           # 32 experts per group
SCALE = 2.5
SC = 2048.0                 # 2^11 lo-part pre-scale

f32 = mybir.dt.float32
f16 = mybir.dt.float16
u32 = mybir.dt.uint32
i32 = mybir.dt.int32
AOT = mybir.AluOpType
ACTF = mybir.ActivationFunctionType

_PROG = None


def _build(stage="full"):
    nc = bacc.Bacc(trn_type="TRN2")
    X = nc.dram_tensor("x", [NM, 128, KT, 2 * MT], f16, kind="ExternalInput")
    Wd = nc.dram_tensor("w", [128, KT, 2 * E], f16, kind="ExternalInput")
    C = nc.dram_tensor("c", [128, 3 * E], f32, kind="ExternalInput")
    OIDX = nc.dram_tensor("oidx", [TLOC, TOP_K], i32, kind="ExternalOutput")
    OW = nc.dram_tensor("ow", [TLOC, TOP_K], f32, kind="ExternalOutput")
    DBG = None
    if stage != "full":
        DBG = nc.dram_tensor("dbg", [TLOC, E], f32, kind="ExternalOutput")

    with TileContext(nc) as tc:
        with (
            tc.tile_pool(name="const", bufs=1) as cpool,
            tc.tile_pool(name="xs", bufs=2) as xpool,
            tc.tile_pool(name="s2", bufs=2) as s2,
            tc.tile_pool(name="psum", bufs=2, space="PSUM") as pspool,
            tc.tile_pool(name="psum1", bufs=1, space="PSUM") as pspool1,
        ):
            c_sb = cpool.tile([128, 3 * E], f32, name="c_sb")
            nc.sync.dma_start(c_sb[:, :], C[:, :])
            b_rep = c_sb[:, 0:E]
            eb_rep = c_sb[:, E:2 * E]
            iota = c_sb[:, 2 * E:3 * E]

            def load_x(m, hf, k0, nk):
                xt = xpool.tile([128, nk * 2 * MT], f16, tag=f"x{hf}",
                                name=f"x{hf}_{m}")
                nc.sync.dma_start(
                    xt.rearrange("p (k t) -> p k t", k=nk),
                    X[m, :, k0:k0 + nk, :],
                )
                return xt

            # Startup DMAs interleaved in the order the k-loop consumes them
            # (the HWDGE queues drain FIFO, so W must not be queued ahead of
            # the first token tile's activations).
            w_sbs = [None] * WC
            x0_off = [sum(X0_CH[:i]) for i in range(len(X0_CH))]
            x_off = [sum(X_CH[:i]) for i in range(len(X_CH))]

            # m0 and m1 run interleaved so the PE has two tiles of work while
            # the (DMA-bound) W load streams in; later tiles go single-file.
            groups = [[0, 1]] + [[m] for m in range(2, NM)]

            # Startup DMAs interleaved in k-consumption order (HWDGE FIFO).
            startup = []
            wi = 0
            xi = {0: 0, 1: 0, 2: 0}
            for k in range(KT):
                while wi < WC and W_OFF[wi] <= k:
                    startup.append(("w", 0, wi)); wi += 1
                while xi[0] < len(X0_CH) and x0_off[xi[0]] <= k:
                    startup.append(("x", 0, xi[0])); xi[0] += 1
                for mm in (1,):
                    while xi[mm] < len(X_CH) and x_off[xi[mm]] <= k:
                        startup.append(("x", mm, xi[mm])); xi[mm] += 1
            x_t = {0: [None] * len(X0_CH), 1: [None] * len(X_CH)}
            for kind, m, i in startup:
                if kind == "w":
                    nk = W_CH[i]
                    wt = cpool.tile([128, nk * 2 * E], f16, name=f"w_sb{i}")
                    nc.sync.dma_start(
                        wt.rearrange("p (k e) -> p k e", k=nk),
                        Wd[:, W_OFF[i]:W_OFF[i] + nk, :],
                    )
                    w_sbs[i] = wt
                else:
                    off = x0_off if m == 0 else x_off
                    ch = X0_CH if m == 0 else X_CH
                    x_t[m][i] = load_x(m, i, off[i], ch[i])

            wmap = []
            for ci, n in enumerate(W_CH):
                wmap += [(ci, j) for j in range(n)]

            def xmap_for(ch):
                mp = []
                for ci, n in enumerate(ch):
                    mp += [(ci, j) for j in range(n)]
                return mp

            def stage2(m, ps_hi, ps_lo):
                lg = s2.tile([128, E], f32, tag="lg", name=f"lg{m}")
                nc.vector.tensor_scalar(lg[:, :], ps_lo[:, :], 1.0 / SC, None,
                                        op0=AOT.mult)
                nc.vector.tensor_add(lg[:, :], lg[:, :], ps_hi[:, :])
                nc.vector.tensor_add(lg[:, :], lg[:, :], b_rep)
                s = s2.tile([128, E], f32, tag="s", name=f"s{m}")
                nc.scalar.activation(s[:, :], lg[:, :], ACTF.Sigmoid)
                sfc = s2.tile([128, E], f32, tag="sfc", name=f"sfc{m}")
                nc.vector.tensor_add(sfc[:, :], s[:, :], eb_rep)

                gmax = s2.tile([128, 8 * N_GROUP], f32, tag="gmax", name=f"gmax{m}")
                for g in range(N_GROUP):
                    nc.vector.max(out=gmax[:, g * 8:(g + 1) * 8],
                                  in_=sfc[:, g * EG:(g + 1) * EG])
                gm3 = gmax.rearrange("p (g c) -> p g c", c=8)
                gs = s2.tile([128, N_GROUP], f32, tag="gs", name=f"gs{m}")
                nc.vector.tensor_add(gs.unsqueeze(2), gm3[:, :, 0:1], gm3[:, :, 1:2])
                g8 = s2.tile([128, 8], f32, tag="g8", name=f"g8{m}")
                nc.vector.max(out=g8[:, :], in_=gs[:, :])
                gmask = s2.tile([128, N_GROUP], f32, tag="gmask", name=f"gmask{m}")
                nc.vector.tensor_scalar(gmask[:, :], gs[:, :],
                                        g8[:, TOPK_GROUP - 1:TOPK_GROUP], None,
                                        op0=AOT.is_ge)
                # top-8 values from the masked per-group maxima (64 wide) --
                # same fp32 values as a 256-wide masked scan, but cheaper.
                gmm = s2.tile([128, 8 * N_GROUP], f32, tag="gmm", name=f"gmm{m}")
                nc.vector.tensor_mul(
                    gmm.rearrange("p (g c) -> p g c", c=8),
                    gm3,
                    gmask.unsqueeze(2).to_broadcast([128, N_GROUP, 8]),
                )
                m8 = s2.tile([128, 8], f32, tag="m8", name=f"m8{m}")
                nc.vector.max(out=m8[:, :], in_=gmm[:, :])

                # Gather unbiased scores by matching top-8 values in sfc
                # (f32 value-match; int converts + ttr hang on hw).
                w8 = s2.tile([128, 8], f32, tag="w8", name=f"w8{m}")
                eqa = s2.tile([128, TOP_K * E], f32, tag="eqa", name=f"eqa{m}")
                scra = s2.tile([128, TOP_K * E], f32, tag="scra", name=f"scra{m}")
                for j in range(TOP_K):
                    nc.vector.tensor_scalar(eqa[:, j * E:(j + 1) * E],
                                            sfc[:, :], m8[:, j:j + 1], None,
                                            op0=AOT.is_equal)
                nc.vector.tensor_mul(
                    scra.rearrange("p (j e) -> p j e", j=TOP_K),
                    eqa.rearrange("p (j e) -> p j e", j=TOP_K),
                    s.unsqueeze(1).to_broadcast([128, TOP_K, E]),
                )
                nc.vector.tensor_reduce(w8[:, :],
                                        scra.rearrange("p (j e) -> p j e", j=TOP_K),
                                        axis=mybir.AxisListType.X, op=AOT.add)
                rs = s2.tile([128, 1], f32, tag="rs", name=f"rs{m}")
                nc.vector.tensor_reduce(rs[:, :], w8[:, :],
                                        axis=mybir.AxisListType.X, op=AOT.add)
                rc = s2.tile([128, 1], f32, tag="rc", name=f"rc{m}")
                nc.vector.reciprocal(rc[:, :], rs[:, :])
                wo = s2.tile([128, 8], f32, tag="wo", name=f"wo{m}")
                nc.vector.tensor_scalar(wo[:, :], w8[:, :], rc[:, 0:1], SCALE,
                                        op0=AOT.mult, op1=AOT.mult)
                nc.sync.dma_start(OW[m * MT:(m + 1) * MT, :], wo[:, :])

                # index extraction (off the critical path -- output only)
                masked = s2.tile([128, E], f32, tag="masked", name=f"masked{m}")
                nc.vector.tensor_mul(
                    masked.rearrange("p (g c) -> p g c", c=EG),
                    sfc.rearrange("p (g c) -> p g c", c=EG),
                    gmask.unsqueeze(2).to_broadcast([128, N_GROUP, EG]),
                )
                i8 = s2.tile([128, 8], u32, tag="i8", name=f"i8{m}")
                nc.vector.max_index(out=i8[:, :], in_max=m8[:, :],
                                    in_values=masked[:, :])
                nc.sync.dma_start(OIDX[m * MT:(m + 1) * MT, :], i8.bitcast(i32))

            for group in groups:
                pss = {}
                for gi, m in enumerate(group):
                    pool = pspool if gi == 0 else pspool1
                    pss[m] = (
                        pool.tile([128, E], f32, tag=f"ps_hi{gi}", name=f"ps_hi{m}"),
                        pool.tile([128, E], f32, tag=f"ps_lo{gi}", name=f"ps_lo{m}"),
                    )
                xts = {}
                for m in group:
                    if m in x_t:
                        xts[m] = (x_t[m], xmap_for(X0_CH if m == 0 else X_CH))
                    else:
                        xts[m] = ([load_x(m, hf, x_off[hf], X_CH[hf])
                                   for hf in range(len(X_CH))], xmap_for(X_CH))
                for k in range(KT):
                    wc, kw = wmap[k]
                    wt = w_sbs[wc]
                    wh = wt[:, kw * 2 * E: kw * 2 * E + E]
                    wl = wt[:, kw * 2 * E + E: (kw + 1) * 2 * E]
                    for m in group:
                        xtl, xmp = xts[m]
                        xc, kl = xmp[k]
                        xt = xtl[xc]
                        xh = xt[:, kl * 2 * MT: kl * 2 * MT + MT]
                        xl = xt[:, kl * 2 * MT + MT: (kl + 1) * 2 * MT]
                        ps_hi, ps_lo = pss[m]
                        nc.tensor.matmul(ps_hi[:, :], xh, wh,
                                         start=(k == 0), stop=(k == KT - 1))
                        nc.tensor.matmul(ps_lo[:, :], xl, wh,
                                         start=(k == 0), stop=False)
                        nc.tensor.matmul(ps_lo[:, :], xh, wl,
                                         start=False, stop=(k == KT - 1))
                for m in group:
                    stage2(m, *pss[m])

    nc.finalize()
    return nc


def _pack_hidden(x_shard: np.ndarray) -> np.ndarray:
    """[TLOC, H] f32 -> [NM, 128, KT, 2*MT] f16 (hi | scaled lo per k-tile)."""
    xT = np.ascontiguousarray(x_shard.T)               # [H, TLOC]
    xh = xT.astype(np.float16)
    xl = ((xT - xh.astype(np.float32)) * SC).astype(np.float16)
    out = np.empty((NM, 128, KT, 2 * MT), np.float16)
    # xh[k*128+p, m*128+t] -> out[m, p, k, t]
    xh4 = xh.reshape(KT, 128, NM, MT).transpose(2, 1, 0, 3)
    xl4 = xl.reshape(KT, 128, NM, MT).transpose(2, 1, 0, 3)
    out[:, :, :, :MT] = xh4
    out[:, :, :, MT:] = xl4
    return np.ascontiguousarray(out)


def _pack_w(W: np.ndarray) -> np.ndarray:
    """[E, H] f32 -> [128, KT, 2*E] f16 (hi | scaled lo)."""
    wT = np.ascontiguousarray(W.T)                     # [H, E]
    wh = wT.astype(np.float16)
    wl = ((wT - wh.astype(np.float32)) * SC).astype(np.float16)
    out = np.empty((128, KT, 2 * E), np.float16)
    out[:, :, :E] = wh.reshape(KT, 128, E).transpose(1, 0, 2)
    out[:, :, E:] = wl.reshape(KT, 128, E).transpose(1, 0, 2)
    return np.ascontiguousarray(out)


def kernel(hidden_states, W, b, e_score_correction_bias):
    global _PROG
    hidden_states = np.asarray(hidden_states, np.float32)
    W = np.asarray(W, np.float32)
    b = np.asarray(b, np.float32)
    eb = np.asarray(e_score_correction_bias, np.float32)

    if _PROG is None:
        _PROG = _build()
    nc = _PROG

    wp = _pack_w(W)
    consts = np.empty((128, 3 * E), np.float32)
    consts[:, 0:E] = b[None, :]
    consts[:, E:2 * E] = eb[None, :]
    consts[:, 2 * E:3 * E] = np.arange(E, dtype=np.float32)[None, :]

    in_maps = []
    for c in range(NCORES):
        shard = hidden_states[c * TLOC:(c + 1) * TLOC]
        in_maps.append({"x": _pack_hidden(shard), "w": wp, "c": consts})

    res = bass_utils.run_bass_kernel_spmd(nc, in_maps, core_ids=list(range(NCORES)))

    idx = np.concatenate([res.results[c]["oidx"] for c in range(NCORES)], axis=0)
    wts = np.concatenate([res.results[c]["ow"] for c in range(NCORES)], axis=0)
    return idx.astype(np.int32), wts.astype(np.float32)
